# revision 1
# baseline (speedup 1.0000x reference)
"""Trainium2 Bass kernel for nn_BfMamba: 2-layer Mamba (selective scan)
over [32, 256, 28, 28] inputs.

Sharding: data-parallel over batch — 8 cores x 4 batch elements each,
parameters replicated. Everything below is self-contained (the grading
harness runs this file alone).

Per-core program layout (per batch element, per layer):
  channel phase (partition dim = channel):
    layernorm stats via ones-matmul on PE, normalize on DVE/ACT
    in_proj / x_proj / dt_proj matmuls on PE (fp32), evictions fused
    into ACT silu/softplus; depthwise conv as shifted scalar_tensor_tensor
  scan phase (partition dim = d_inner tile of 128, free dim = L=784):
    per state index s (16): dA_s = exp(dt * A[:,s]) on ACT (per-partition
    scale), b_s = dtx * B_s (B_s broadcast to 128 partitions via a
    DRAM-bounce DMA), h_s = tensor_tensor_scan(dA_s, b_s), and
    y += h_s * C_s on DVE
  epilogue: y = (y + xc*D) * silu(z), out_proj matmul, result overwrites
  the persistent x tile in SBUF; final DMA to DRAM.
"""
import time
from contextlib import ExitStack

import numpy as np

import bass_rust
import orjson as _orjson

import concourse.bass as bass
import concourse.tile as tile
from concourse import mybir
from concourse import bass2jax
from concourse.vector_clock import ScopedClock

# ----------------------------------------------------------------------------
# Workarounds for this walrus build (rejects >1 sync wait per instruction).
# ----------------------------------------------------------------------------


def _patched_drain_and_barrier(self, tick_clock, wait_clock):
    nc = self.nc
    dummy = nc.sync.nop()
    wait_clock.add_sem_waits(dummy.ins, ScopedClock({None: tick_clock.global_clock}))
    si = dummy.ins.sync_info
    waits = list(si.on_wait) if si else []
    if len(waits) > 1:
        dummy.ins.sync_info = bass_rust.SyncInfo(
            on_wait=[waits[0]], on_update=list(si.on_update))
        for w in waits[1:]:
            n2 = nc.sync.nop()
            n2.ins.sync_info = bass_rust.SyncInfo(on_wait=[w], on_update=[])
    nc.sync.drain()
    nc.all_engine_barrier()
    assert self.sems is not None
    popped = nc._tile_sem_poison_stack.pop()
    assert popped is self._sem_poison
    nc.clear_and_free_semaphores(list(self.sems.allocated().values()))
    nc.all_engine_barrier()


tile.TileContext._drain_and_barrier = _patched_drain_and_barrier

_MSW_CTR = [0]


def _split_multiwait_bir(bir_json: bytes) -> bytes:
    d = _orjson.loads(bir_json)
    changed = False
    for fn in d.get("functions", []):
        for bb in fn.get("blocks", []):
            new = None
            insts = bb.get("instructions", [])
            for idx, ins in enumerate(insts):
                si = ins.get("sync_info")
                waits = si.get("on_wait") if si else None
                if waits and len(waits) > 1 and ins.get("engine") != "Unassigned":
                    if new is None:
                        new = list(insts[:idx])
                    for w in waits[:-1]:
                        _MSW_CTR[0] += 1
                        nop = {
                            "engine": ins["engine"], "ins": [], "outs": [],
                            "name": f"I-msw{_MSW_CTR[0]}", "opcode": "NoOp",
                            "sync_info": {"on_update": [], "on_wait": [w]},
                        }
                        if "debug" in ins:
                            nop["debug"] = ins["debug"]
                        new.append(nop)
                    si["on_wait"] = [waits[-1]]
                    changed = True
                if new is not None:
                    new.append(ins)
            if new is not None:
                bb["instructions"] = new
    return _orjson.dumps(d) if changed else bir_json


_orig_compile_bir_kernel = bass2jax.compile_bir_kernel


def _patched_compile_bir_kernel(bir_json, tmpdir, neff_name="file.neff"):
    return _orig_compile_bir_kernel(
        _split_multiwait_bir(bir_json), tmpdir, neff_name=neff_name)


bass2jax.compile_bir_kernel = _patched_compile_bir_kernel

# ----------------------------------------------------------------------------
# Problem constants
# ----------------------------------------------------------------------------
B_SZ, CH, H, W = 32, 256, 28, 28
L = H * W                      # 784
D_INNER, D_STATE, D_CONV, DT_RANK, DEPTH = 512, 16, 4, 16, 2
N_CORES = 8
BPC = B_SZ // N_CORES          # batch per core = 4
NDT = D_INNER // 128           # d_inner tiles = 4
NCT = CH // 128                # channel tiles = 2
NC2 = L // 2                   # 392, matmul N-chunk (1 PSUM bank)

F32 = mybir.dt.float32
F16 = mybir.dt.float16

import os
ABLATE = set(os.environ.get("KERNEL_ABLATE", "").split(","))

# scan-phase dtype knobs
BC_DT = F16    # dtype of broadcast B/C tiles
DTX_DT = F16   # dtype of dtx
B_DT = F16     # dtype of b_s = dtx*B_s
HS_DT = F16    # dtype of scan output h_s (state stays fp32 internally)
PS_DT = F16    # dtype of p_s = h_s*C_s
ZS_DT = F32    # dtype of silu(z)


def build_nc(repeats=1, tiny_out=False):
    nc = bass.Bass()
    x_in = nc.declare_dram_parameter("x_in", [BPC, CH, L], F32, isOutput=False)
    nw = nc.declare_dram_parameter("nw", [DEPTH, 128, NCT], F32, isOutput=False)
    nb = nc.declare_dram_parameter("nb", [DEPTH, 128, NCT], F32, isOutput=False)
    w_in_T = nc.declare_dram_parameter("w_in_T", [DEPTH, NCT, 128, 2 * D_INNER],
                                       F32, isOutput=False)
    conv_w = nc.declare_dram_parameter("conv_w", [DEPTH, NDT, 128, D_CONV],
                                       F32, isOutput=False)
    conv_b = nc.declare_dram_parameter("conv_b", [DEPTH, NDT, 128, 1],
                                       F32, isOutput=False)
    n_conv_b = nc.declare_dram_parameter("n_conv_b", [DEPTH, NDT, 128, 1],
                                         F32, isOutput=False)
    w_x_T = nc.declare_dram_parameter("w_x_T", [DEPTH, NDT, 128, 48],
                                      F32, isOutput=False)
    w_dt_T = nc.declare_dram_parameter("w_dt_T", [DEPTH, DT_RANK, D_INNER],
                                       F32, isOutput=False)
    dt_b = nc.declare_dram_parameter("dt_b", [DEPTH, NDT, 128, 1],
                                     F32, isOutput=False)
    a_s = nc.declare_dram_parameter("a_s", [DEPTH, NDT, 128, D_STATE],
                                    F32, isOutput=False)
    d_p = nc.declare_dram_parameter("d_p", [DEPTH, NDT, 128, 1],
                                    F32, isOutput=False)
    w_out_T = nc.declare_dram_parameter("w_out_T", [DEPTH, NDT, 128, CH],
                                        F32, isOutput=False)
    eye_in = nc.declare_dram_parameter("eye", [128, 128], F16, isOutput=False)
    y_shape = [1, 16] if tiny_out else [BPC, CH, L]
    y_out = nc.declare_dram_parameter("y_out", y_shape, F32, isOutput=True)

    with tile.TileContext(nc) as tc, ExitStack() as ctx:
        pool = ctx.enter_context(tc.tile_pool(name="sbuf", bufs=1))
        wpool = ctx.enter_context(tc.tile_pool(name="wts", bufs=1))
        tpool = ctx.enter_context(tc.tile_pool(name="tmp", bufs=1))
        cpool = ctx.enter_context(tc.tile_pool(name="cube", bufs=2))
        bcpool = ctx.enter_context(tc.tile_pool(name="bcast", bufs=2))
        psum = ctx.enter_context(tc.tile_pool(name="psum", bufs=3, space="PSUM"))
        psum1 = ctx.enter_context(tc.tile_pool(name="psum1", bufs=1, space="PSUM"))
        dram = ctx.enter_context(tc.tile_pool(name="dram", bufs=2, space="DRAM"))

        ones = pool.tile([128, 1], F32, tag="ones", name="ones")
        nc.vector.memset(ones[:], 1.0)
        ones_row = pool.tile([1, 128], F32, tag="ones_row", name="ones_row")
        nc.vector.memset(ones_row[:], 1.0)
        eps_t = pool.tile([128, 1], F32, tag="eps", name="eps")
        nc.vector.memset(eps_t[:], 1e-5)
        eye_sb = pool.tile([128, 128], F16, tag="eye", name="eye")
        nc.sync.dma_start(eye_sb[:], eye_in[:])

        # inter-layer activations bounce through DRAM
        x_dr = [dram.tile([CH, L], F32, tag=f"xdr{b}", name=f"xdr{b}")
                for b in range(BPC)]

        for rep in range(repeats):
            for layer in range(DEPTH):
                # ---- load layer weights ----
                nw_sb = wpool.tile([128, NCT], F32, tag="nw", name="nw")
                nc.sync.dma_start(nw_sb[:], nw[layer])
                nb_sb = wpool.tile([128, NCT], F32, tag="nb", name="nb")
                nc.sync.dma_start(nb_sb[:], nb[layer])
                win_sb = [wpool.tile([128, 2 * D_INNER], F32, tag=f"win{ct}", name=f"win{ct}")
                          for ct in range(NCT)]
                for ct in range(NCT):
                    nc.sync.dma_start(win_sb[ct][:], w_in_T[layer, ct])
                cw_sb = [wpool.tile([128, D_CONV], F32, tag=f"cw{m}", name=f"cw{m}")
                         for m in range(NDT)]
                cb_sb = [wpool.tile([128, 1], F32, tag=f"cb{m}", name=f"cb{m}") for m in range(NDT)]
                ncb_sb = [wpool.tile([128, 1], F32, tag=f"ncb{m}", name=f"ncb{m}") for m in range(NDT)]
                wx_sb = [wpool.tile([128, 48], F32, tag=f"wx{m}", name=f"wx{m}") for m in range(NDT)]
                dtb_sb = [wpool.tile([128, 1], F32, tag=f"dtb{m}", name=f"dtb{m}") for m in range(NDT)]
                as_sb = [wpool.tile([128, D_STATE], F32, tag=f"as{m}", name=f"as{m}")
                         for m in range(NDT)]
                dp_sb = [wpool.tile([128, 1], F32, tag=f"dp{m}", name=f"dp{m}") for m in range(NDT)]
                wout_sb = [wpool.tile([128, CH], F32, tag=f"wout{m}", name=f"wout{m}")
                           for m in range(NDT)]
                for m in range(NDT):
                    nc.sync.dma_start(cw_sb[m][:], conv_w[layer, m])
                    nc.sync.dma_start(cb_sb[m][:], conv_b[layer, m])
                    nc.sync.dma_start(ncb_sb[m][:], n_conv_b[layer, m])
                    nc.sync.dma_start(wx_sb[m][:], w_x_T[layer, m])
                    nc.sync.dma_start(dtb_sb[m][:], dt_b[layer, m])
                    nc.sync.dma_start(as_sb[m][:], a_s[layer, m])
                    nc.sync.dma_start(dp_sb[m][:], d_p[layer, m])
                    nc.sync.dma_start(wout_sb[m][:], w_out_T[layer, m])
                wdt_sb = wpool.tile([DT_RANK, D_INNER], F32, tag="wdt", name="wdt")
                nc.sync.dma_start(wdt_sb[:], w_dt_T[layer])

                # ---- per-batch: load x, LN stats, broadcast mu/inv ----
                first_in = (rep == 0 and layer == 0)
                x_cur = {}
                for b in range(BPC):
                    xc_t = [tpool.tile([128, L], F32, tag=f"xcur{ct}",
                                       name=f"xcur{ct}")
                            for ct in range(NCT)]
                    x_cur[b] = xc_t
                    for ct in range(NCT):
                        src_ap = (x_in[b, ct * 128:(ct + 1) * 128, :] if first_in
                                  else x_dr[b][ct * 128:(ct + 1) * 128, :])
                        nc.sync.dma_start(xc_t[ct][:], src_ap)
                    st0 = tpool.tile([1, L], F32, tag="st0", name="st0")
                    st1 = tpool.tile([1, L], F32, tag="st1", name="st1")
                    x2s = []
                    for ct in range(NCT):
                        x2 = tpool.tile([128, L], F32, tag=f"xn{ct}", name=f"xn{ct}")
                        nc.scalar.square(x2[:], xc_t[ct][:])
                        x2s.append(x2)
                    for nch in range(2):
                        sl = slice(nch * NC2, (nch + 1) * NC2)
                        ssum = psum.tile([1, NC2], F32, tag="mm", name="ssum")
                        ssq = psum.tile([1, NC2], F32, tag="mm", name="ssq")
                        for ct in range(NCT):
                            nc.tensor.matmul(ssum[:], ones[:], xc_t[ct][:, sl],
                                             start=(ct == 0), stop=(ct == NCT - 1))
                            nc.tensor.matmul(ssq[:], ones[:], x2s[ct][:, sl],
                                             start=(ct == 0), stop=(ct == NCT - 1))
                        nc.scalar.copy(st0[0:1, sl], ssum[:])
                        nc.scalar.copy(st1[0:1, sl], ssq[:])
                    mu_v = tpool.tile([1, L], F32, tag="muv", name="muv")
                    inv_v = tpool.tile([1, L], F32, tag="invv", name="invv")
                    lnt = tpool.tile([1, L], F32, tag="lnt", name="lnt")
                    nc.vector.tensor_scalar_mul(mu_v[:], st0[0:1, :], 1.0 / CH)
                    nc.vector.tensor_scalar_mul(inv_v[:], st1[0:1, :], 1.0 / CH)
                    nc.vector.tensor_mul(lnt[:], mu_v[:], mu_v[:])
                    nc.vector.tensor_sub(inv_v[:], inv_v[:], lnt[:])
                    nc.scalar.activation(inv_v[:], inv_v[:],
                                         mybir.ActivationFunctionType.Ln,
                                         bias=eps_t[0:1, 0:1])
                    nc.scalar.activation(inv_v[:], inv_v[:],
                                         mybir.ActivationFunctionType.Exp,
                                         scale=-0.5)

                    # broadcast mu, inv to 128 partitions via K=1 matmul
                    mub = tpool.tile([128, L], F32, tag="mub", name="mub")
                    invb = tpool.tile([128, L], F32, tag="invb", name="invb")
                    for nch in range(2):
                        sl = slice(nch * NC2, (nch + 1) * NC2)
                        bc_ps = psum.tile([128, NC2], F32, tag="mm", name="ssum")
                        nc.tensor.matmul(bc_ps[:], ones_row[:], mu_v[0:1, sl],
                                         start=True, stop=True)
                        nc.scalar.copy(mub[:, sl], bc_ps[:])
                        bc_ps = psum.tile([128, NC2], F32, tag="mm", name="ssq")
                        nc.tensor.matmul(bc_ps[:], ones_row[:], inv_v[0:1, sl],
                                         start=True, stop=True)
                        nc.scalar.copy(invb[:, sl], bc_ps[:])

                    # normalize into xn [ct][128, L]
                    xn = [tpool.tile([128, L], F32, tag=f"xn{ct}", name=f"xn{ct}")
                          for ct in range(NCT)]
                    for ct in range(NCT):
                        nc.vector.tensor_sub(xn[ct][:], x_cur[b][ct][:], mub[:])
                        nc.vector.tensor_mul(xn[ct][:], xn[ct][:], invb[:])
                        nc.scalar.activation(xn[ct][:], xn[ct][:],
                                             mybir.ActivationFunctionType.Identity,
                                             bias=nb_sb[:, ct:ct + 1],
                                             scale=nw_sb[:, ct:ct + 1])

                    # ---- in_proj: xz[e, l], e in 8 tiles of 128 ----
                    xi = [tpool.tile([128, D_CONV - 1 + L], F32, tag=f"xi{m}", name=f"xi{m}")
                          for m in range(NDT)]
                    zs = [tpool.tile([128, L], ZS_DT, tag=f"zs{m}", name=f"zs{m}")
                          for m in range(NDT)]
                    for m in range(NDT):
                        nc.vector.memset(xi[m][:, 0:D_CONV - 1], 0.0)
                    for e in range(2 * D_INNER // 128):
                        for nch in range(2):
                            sl = slice(nch * NC2, (nch + 1) * NC2)
                            mm = psum.tile([128, NC2], F32, tag="mm", name="mm")
                            for ct in range(NCT):
                                nc.tensor.matmul(
                                    mm[:], win_sb[ct][:, e * 128:(e + 1) * 128],
                                    xn[ct][:, sl],
                                    start=(ct == 0), stop=(ct == NCT - 1))
                            if e < NDT:
                                out_ap = xi[e][:, D_CONV - 1 + nch * NC2:
                                               D_CONV - 1 + (nch + 1) * NC2]
                                nc.scalar.copy(out_ap, mm[:])
                            else:
                                zcp = tpool.tile([128, NC2], F32, tag="zcp",
                                                 name="zcp")
                                nc.scalar.copy(zcp[:], mm[:])
                                sig = tpool.tile([128, NC2], F32, tag="sig",
                                                 name="sig")
                                nc.scalar.activation(
                                    sig[:], zcp[:],
                                    mybir.ActivationFunctionType.Exp, scale=-1.0)
                                nc.scalar.activation(
                                    sig[:], sig[:],
                                    mybir.ActivationFunctionType.Ln,
                                    bias=ones[:, 0:1])
                                nc.scalar.activation(
                                    sig[:], sig[:],
                                    mybir.ActivationFunctionType.Exp, scale=-1.0)
                                nc.vector.tensor_mul(zs[e - NDT][:, sl],
                                                     zcp[:], sig[:])

                    # ---- depthwise causal conv + silu -> xc ----
                    xc = [tpool.tile([128, L], F32, tag=f"xc{m}", name=f"xc{m}")
                          for m in range(NDT)]
                    for m in range(NDT):
                        acc = tpool.tile([128, L], F32, tag="cacc", name="cacc")
                        nc.vector.tensor_scalar_mul(acc[:], xi[m][:, 0:L],
                                                    cw_sb[m][:, 0:1])
                        for k in range(1, D_CONV):
                            nc.vector.scalar_tensor_tensor(
                                acc[:], xi[m][:, k:k + L], cw_sb[m][:, k:k + 1],
                                acc[:], mybir.AluOpType.mult, mybir.AluOpType.add)
                        sigc = tpool.tile([128, L], F32, tag="sigc",
                                          name="sigc")
                        nc.scalar.activation(sigc[:], acc[:],
                                             mybir.ActivationFunctionType.Exp,
                                             scale=-1.0, bias=ncb_sb[m][:, 0:1])
                        nc.scalar.activation(sigc[:], sigc[:],
                                             mybir.ActivationFunctionType.Ln,
                                             bias=ones[:, 0:1])
                        nc.scalar.activation(sigc[:], sigc[:],
                                             mybir.ActivationFunctionType.Exp,
                                             scale=-1.0)
                        nc.vector.scalar_tensor_tensor(
                            xc[m][:], acc[:], cb_sb[m][:, 0:1], sigc[:],
                            mybir.AluOpType.add, mybir.AluOpType.mult)

                    # ---- x_proj -> x_dbl [48, L] (one PSUM bank per chunk) ----
                    xdall = tpool.tile([48, L], BC_DT, tag="xdall", name="xdall")
                    dtr_sb = tpool.tile([DT_RANK, L], F32, tag="dtr", name="dtr")
                    for nch in range(2):
                        sl = slice(nch * NC2, (nch + 1) * NC2)
                        xd_ps = psum.tile([128, NC2], F32, tag="mm", name="xd")
                        for m in range(NDT):
                            nc.tensor.matmul(xd_ps[0:48, :], wx_sb[m][:],
                                             xc[m][:, sl],
                                             start=(m == 0), stop=(m == NDT - 1))
                        nc.scalar.copy(xdall[:, sl], xd_ps[0:48, :])
                        nc.scalar.copy(dtr_sb[:, sl], xd_ps[0:DT_RANK, :])
                    # bounce B/C rows through DRAM for partition broadcast
                    bc_dr = dram.tile([2 * D_STATE, L], BC_DT, tag="bcd", name="bcd")
                    nc.sync.dma_start(bc_dr[:], xdall[DT_RANK:48, :])

                    # ---- dt = softplus(dt_proj @ dt_r + bias); dtx = dt*xc ----
                    dt_sb = [tpool.tile([128, L], F32, tag=f"dt{m}", name=f"dt{m}")
                             for m in range(NDT)]
                    dtx = [tpool.tile([128, L], DTX_DT, tag=f"dtx{m}", name=f"dtx{m}")
                           for m in range(NDT)]
                    for m in range(NDT):
                        for nch in range(2):
                            sl = slice(nch * NC2, (nch + 1) * NC2)
                            mm = psum.tile([128, NC2], F32, tag="mm", name="mm")
                            nc.tensor.matmul(mm[:],
                                             wdt_sb[:, m * 128:(m + 1) * 128],
                                             dtr_sb[:, sl], start=True, stop=True)
                            nc.scalar.activation(
                                dt_sb[m][:, sl], mm[:],
                                mybir.ActivationFunctionType.Exp,
                                bias=dtb_sb[m][:, 0:1])
                            nc.scalar.activation(
                                dt_sb[m][:, sl], dt_sb[m][:, sl],
                                mybir.ActivationFunctionType.Ln,
                                bias=ones[:, 0:1])
                        nc.vector.tensor_mul(dtx[m][:], dt_sb[m][:], xc[m][:])

                    # ---- scan phase (two m-groups to fit PSUM) ----
                    y_ps = {}
                    for mg in range(2):
                        ms = (2 * mg, 2 * mg + 1)
                        for m in ms:
                            y_ps[m] = [psum.tile([128, NC2], F32, tag="yps",
                                                 name=f"yps{m}_{nch}", bufs=4)
                                       for nch in range(2)]
                        for s in range(D_STATE):
                            bb = bcpool.tile([128, L], BC_DT, tag="bb", name="bb")
                            src_ap = bass.AP(bc_dr[:].tensor,
                                             bc_dr[s:s + 1, :].offset,
                                             [[0, 128], [1, L]])
                            nc.sync.dma_start(bb[:], src_ap)
                            cb2 = bcpool.tile([128, L], BC_DT, tag="cb2",
                                              name="cb2")
                            src_ap = bass.AP(
                                bc_dr[:].tensor,
                                bc_dr[D_STATE + s:D_STATE + s + 1, :].offset,
                                [[0, 128], [1, L]])
                            nc.sync.dma_start(cb2[:], src_ap)
                            for m in ms:
                                da = cpool.tile([128, L], F32, tag="da",
                                                name="da")
                                if "exp" not in ABLATE:
                                    nc.scalar.activation(
                                        da[:], dt_sb[m][:],
                                        mybir.ActivationFunctionType.Exp,
                                        scale=as_sb[m][:, s:s + 1])
                                if "bmul" not in ABLATE:
                                    bs = cpool.tile([128, L], B_DT, tag="bs",
                                                    name="bs")
                                    nc.vector.tensor_mul(bs[:], dtx[m][:], bb[:])
                                    scan_in = bs
                                else:
                                    scan_in = dtx[m]
                                if "scan" not in ABLATE:
                                    hs = cpool.tile([128, L], HS_DT, tag="hs",
                                                    name="hs")
                                    nc.vector.tensor_tensor_scan(
                                        hs[:], da[:], scan_in[:], 0.0,
                                        mybir.AluOpType.mult,
                                        mybir.AluOpType.add)
                                else:
                                    hs = scan_in
                                if "ymul" not in ABLATE:
                                    ps = cpool.tile([128, L], PS_DT, tag="psx",
                                                    name="ps")
                                    nc.vector.tensor_mul(ps[:], hs[:], cb2[:])
                                    for nch in range(2):
                                        sl = slice(nch * NC2, (nch + 1) * NC2)
                                        nc.tensor.matmul(
                                            y_ps[m][nch][:], eye_sb[:],
                                            ps[:, sl],
                                            start=(s == 0),
                                            stop=(s == D_STATE - 1))

                    # ---- epilogue: skip, gate, out_proj ----
                    g = [tpool.tile([128, L], F32, tag=f"g{m}", name=f"g{m}")
                         for m in range(NDT)]
                    for m in range(NDT):
                        for nch in range(2):
                            sl = slice(nch * NC2, (nch + 1) * NC2)
                            nc.vector.scalar_tensor_tensor(
                                g[m][:, sl], xc[m][:, sl], dp_sb[m][:, 0:1],
                                y_ps[m][nch][:],
                                mybir.AluOpType.mult, mybir.AluOpType.add)
                        nc.vector.tensor_mul(g[m][:], g[m][:], zs[m][:])
                    last = (rep == repeats - 1 and layer == DEPTH - 1)
                    for ct in range(NCT):
                        stage = tpool.tile([128, L], F32,
                                           tag=("mub" if ct == 0 else "invb"),
                                           name=f"stage{ct}")
                        for nch in range(2):
                            sl = slice(nch * NC2, (nch + 1) * NC2)
                            mm = psum.tile([128, NC2], F32, tag="mm", name="mm")
                            for m in range(NDT):
                                nc.tensor.matmul(
                                    mm[:], wout_sb[m][:, ct * 128:(ct + 1) * 128],
                                    g[m][:, sl],
                                    start=(m == 0), stop=(m == NDT - 1))
                            nc.scalar.copy(stage[:, sl], mm[:])
                        if last and tiny_out:
                            nc.sync.dma_start(
                                x_dr[b][ct * 128:(ct + 1) * 128, :], stage[:])
                            if b == 0 and ct == 0:
                                nc.sync.dma_start(y_out[:], stage[0:1, 0:16])
                        else:
                            dst = (y_out[b, ct * 128:(ct + 1) * 128, :] if last
                                   else x_dr[b][ct * 128:(ct + 1) * 128, :])
                            nc.sync.dma_start(dst, stage[:])

    return nc


# ----------------------------------------------------------------------------
# Host-side prep + execution
# ----------------------------------------------------------------------------

def prep_params(inputs):
    """Rearrange reference parameters into the kernel's layouts."""
    p = {}
    p["nw"] = np.ascontiguousarray(
        inputs["norm_w"].reshape(DEPTH, NCT, 128).transpose(0, 2, 1)).astype(np.float32)
    p["nb"] = np.ascontiguousarray(
        inputs["norm_b"].reshape(DEPTH, NCT, 128).transpose(0, 2, 1)).astype(np.float32)
    # in_proj_w [l, 2*D_INNER, CH] -> [l, ct, 128c, 2*D_INNER]
    w = np.transpose(inputs["in_proj_w"], (0, 2, 1))  # [l, CH, 2D]
    p["w_in_T"] = np.ascontiguousarray(
        w.reshape(DEPTH, NCT, 128, 2 * D_INNER)).astype(np.float32)
    p["conv_w"] = np.ascontiguousarray(
        inputs["conv_w"].reshape(DEPTH, NDT, 128, D_CONV)).astype(np.float32)
    p["conv_b"] = np.ascontiguousarray(
        inputs["conv_b"].reshape(DEPTH, NDT, 128, 1)).astype(np.float32)
    p["n_conv_b"] = -p["conv_b"]
    # x_proj_w [l, 48, D_INNER] -> [l, m, 128d, 48]
    w = np.transpose(inputs["x_proj_w"], (0, 2, 1))   # [l, D_INNER, 48]
    p["w_x_T"] = np.ascontiguousarray(
        w.reshape(DEPTH, NDT, 128, 48)).astype(np.float32)
    # dt_proj_w [l, D_INNER, DT_RANK] -> [l, r, D_INNER]
    p["w_dt_T"] = np.ascontiguousarray(
        np.transpose(inputs["dt_proj_w"], (0, 2, 1))).astype(np.float32)
    p["dt_b"] = np.ascontiguousarray(
        inputs["dt_proj_b"].reshape(DEPTH, NDT, 128, 1)).astype(np.float32)
    p["a_s"] = np.ascontiguousarray(
        (-np.exp(inputs["A_log"])).reshape(DEPTH, NDT, 128, D_STATE)).astype(np.float32)
    p["d_p"] = np.ascontiguousarray(
        inputs["D_param"].reshape(DEPTH, NDT, 128, 1)).astype(np.float32)
    p["eye"] = np.eye(128, dtype=np.float16)
    # out_proj_w [l, CH, D_INNER] -> [l, m, 128d, CH]
    w = np.transpose(inputs["out_proj_w"], (0, 2, 1))  # [l, D_INNER, CH]
    p["w_out_T"] = np.ascontiguousarray(
        w.reshape(DEPTH, NDT, 128, CH)).astype(np.float32)
    return p


_RUNNER_CACHE = {}


def _get_runner(repeats=1, reduced=False):
    import jax
    from jax.sharding import Mesh, PartitionSpec
    from jax.experimental.shard_map import shard_map
    from concourse.bass2jax import _bass_exec_p, install_neuronx_cc_hook

    key = (repeats, reduced)
    if key in _RUNNER_CACHE:
        return _RUNNER_CACHE[key]
    install_neuronx_cc_hook()
    nc = build_nc(repeats, tiny_out=reduced)
    partition_name = (nc.partition_id_tensor.name
                      if nc.partition_id_tensor else None)
    in_names, out_names, out_avals, zero_outs = [], [], [], []
    for alloc in nc.m.functions[0].allocations:
        if not isinstance(alloc, mybir.MemoryLocationSet):
            continue
        name = alloc.memorylocations[0].name
        if alloc.kind == "ExternalInput":
            if name != partition_name:
                in_names.append(name)
        elif alloc.kind == "ExternalOutput":
            shape = tuple(alloc.tensor_shape)
            dtype = mybir.dt.np(alloc.dtype)
            out_names.append(name)
            out_avals.append(jax.core.ShapedArray(shape, dtype))
            zero_outs.append(np.zeros(shape, dtype))
    n_params = len(in_names)
    all_in_names = in_names + out_names
    if partition_name is not None:
        all_in_names.append(partition_name)

    def _body(*args):
        operands = list(args)
        if partition_name is not None:
            operands.append(bass2jax.partition_id_tensor())
        outs = _bass_exec_p.bind(
            *operands,
            out_avals=tuple(out_avals),
            in_names=tuple(all_in_names),
            out_names=tuple(out_names),
            lowering_input_output_aliases=(),
            sim_require_finite=False,
            sim_require_nnan=False,
            nc=nc,
        )
        return tuple(outs)

    devices = jax.devices()[:N_CORES]
    mesh = Mesh(np.asarray(devices), ("core",))
    in_specs = (PartitionSpec("core"),) * (n_params + len(out_names))
    out_specs = (PartitionSpec("core"),) * len(out_names)
    sharded = jax.jit(shard_map(_body, mesh=mesh, in_specs=in_specs,
                                out_specs=out_specs, check_rep=False))

    def prep(in_maps):
        per_core = [[np.asarray(m[nm]) for nm in in_names] for m in in_maps]
        concat_in = [np.concatenate([per_core[c][i] for c in range(N_CORES)],
                                    axis=0) for i in range(n_params)]
        concat_zeros = [np.zeros((N_CORES * z.shape[0], *z.shape[1:]), z.dtype)
                        for z in zero_outs]
        return [jax.device_put(a) for a in concat_in + concat_zeros]

    def run_dev(dev_args):
        out_arrs = sharded(*dev_args)
        jax.block_until_ready(out_arrs)
        return out_arrs

    def run(in_maps):
        out_arrs = run_dev(prep(in_maps))
        out_arrs = [np.asarray(a) for a in out_arrs]
        if reduced:
            return out_arrs
        return [
            {nm: out_arrs[i].reshape(N_CORES, *out_avals[i].shape)[c]
             for i, nm in enumerate(out_names)}
            for c in range(N_CORES)
        ]

    run.prep = prep
    run.run_dev = run_dev
    _RUNNER_CACHE[key] = run
    return run


def kernel(**inputs) -> np.ndarray:
    x = np.asarray(inputs["bbox_feats"], dtype=np.float32)
    p = prep_params({k: np.asarray(v) for k, v in inputs.items()})
    run = _get_runner(1)
    in_maps = []
    for c in range(N_CORES):
        m = dict(p)
        m["x_in"] = np.ascontiguousarray(
            x[c * BPC:(c + 1) * BPC].reshape(BPC, CH, L))
        in_maps.append(m)
    res = run(in_maps)
    out = np.concatenate([res[c]["y_out"] for c in range(N_CORES)], axis=0)
    return out.reshape(B_SZ, CH, H, W).astype(np.float32)


def run_timed(inputs, repeats, reps=15):
    """Time the kernel with `repeats` internal iterations: inputs stay
    on-device, outputs reduced to scalars so wall time ~= dispatch + exec."""
    x = np.asarray(inputs["bbox_feats"], dtype=np.float32)
    p = prep_params({k: np.asarray(v) for k, v in inputs.items()})
    run = _get_runner(repeats, reduced=True)
    in_maps = []
    for c in range(N_CORES):
        m = dict(p)
        m["x_in"] = np.ascontiguousarray(
            x[c * BPC:(c + 1) * BPC].reshape(BPC, CH, L))
        in_maps.append(m)
    dev_args = run.prep(in_maps)
    run.run_dev(dev_args)  # compile+warm
    ts = []
    for _ in range(reps):
        t0 = time.perf_counter()
        run.run_dev(dev_args)
        ts.append(time.perf_counter() - t0)
    return min(ts)

